# revision 34
# baseline (speedup 1.0000x reference)
"""Causal multi-head attention (B=128, T=256, C=384, H=6, Dh=64) on 8 TRN2
NeuronCores, data-parallel over batch (16 batches per core, no collectives).

Layout strategy per core (v2 — transposed-scores):
  - host pre-transposes x to xT [b, C, T] and casts activations/weights to bf16
  - QT/KT computed as [D, T] (Dh on partitions)
  - scores are computed TRANSPOSED: ST[ts, tq] = KT_h^T @ QT_h, so
    PT = exp(ST)*mask feeds the AV matmul directly as the stationary
    operand — no PE transposes of P at all
  - V stored per head with an appended ones column ([ts, 65]); the AV
    matmul out = PT^T @ [V_h | 1] lands O[tq, 0:64] AND the softmax
    denominators in col 64 of the same PSUM tile — row sums are free
  - normalization is a per-partition tensor_scalar_mul fused into the
    PSUM->SBUF move (exp is never max-subtracted; scores are O(30) so
    fp32 exp cannot overflow); one reciprocal serves a head pair via a
    packed PSUM tile
  - O [tq, D] is transposed on the PE (6 [128,128] blocks per batch) to
    OT for the output projection, which consumes OT as stationary; y is
    stored bf16 and upcast on the host
  - the whole kernel is one flat software pipeline over (group, batch,
    head) units: AV/normalize lag scores/exp/mask by two units, and the
    next group's QKV-projection chunks are interleaved one-per-unit so
    the PE never drains between groups (~98% TensorMatrix occupancy);
    the causal mask multiply is split across vector (ts0 x tq0 block)
    and gpsimd (ts1 x tq1) so it stays off the scalar/vector hot paths
"""

import sys

sys.path.insert(0, "/opt/trn_rl_repo")

import numpy as np
import ml_dtypes

import concourse.bass as bass
import concourse.tile as tile
from concourse import mybir
from concourse.bass_utils import run_bass_kernel_spmd
from concourse.masks import make_identity


def split_multi_waits(nc):
    """This walrus build accepts at most one sync-wait command per
    instruction; hoist extra waits into standalone InstEventSemaphore
    instructions on the same engine queue (queue waits run in order before
    the original instruction, so semantics are preserved)."""
    ctr = [0]

    def mk(engine, wait):
        ctr[0] += 1
        return mybir.InstEventSemaphore(
            name=f"WSPLIT-{ctr[0]}",
            engine=engine,
            ins=[],
            outs=[],
            sync_info=mybir.SyncInfo(on_wait=[wait], on_update=[]),
        )

    for f in nc.m.functions:
        for blk in f.blocks:
            insts = blk.instructions
            out = []
            for inst in insts:
                si = inst.sync_info
                if si is not None and len(si.on_wait) > 1:
                    waits = list(si.on_wait)
                    for w in waits[:-1]:
                        out.append(mk(inst.engine, w))
                    inst.sync_info = mybir.SyncInfo(
                        on_wait=[waits[-1]], on_update=list(si.on_update)
                    )
                out.append(inst)
            insts[:] = out
    return nc


N_CORES = 8
B, T, C = 128, 256, 384
H, DH = 6, 64
BL = B // N_CORES  # batches per core
GB = 2  # batches per projection group (N = GB*T = 512 <= one PSUM bank fp32)
NG = BL // GB
BF16 = mybir.dt.bfloat16
FP32 = mybir.dt.float32
AFT = mybir.ActivationFunctionType
SCALE = DH**-0.5  # 0.125


def build_kernel() -> bass.Bass:
    nc = bass.Bass()
    xT = nc.dram_tensor("xT", [BL, C, T], BF16, kind="ExternalInput")
    wqt = nc.dram_tensor("wqt", [C, C], BF16, kind="ExternalInput")  # Wq.T [C, D]
    wkt = nc.dram_tensor("wkt", [C, C], BF16, kind="ExternalInput")
    wvt = nc.dram_tensor("wvt", [C, C], BF16, kind="ExternalInput")
    wot = nc.dram_tensor("wot", [C, C], BF16, kind="ExternalInput")  # Wo.T [D, C]
    y = nc.dram_tensor("y", [BL, T, C], BF16, kind="ExternalOutput")

    with tile.TileContext(nc) as tc:
        with (
            tc.tile_pool(name="const", bufs=1) as const,
            tc.tile_pool(name="xp", bufs=NG) as xp,
            tc.tile_pool(name="qkv", bufs=4) as qkv,
            tc.tile_pool(name="vp", bufs=8) as vp,
            tc.tile_pool(name="pp", bufs=8) as pp,
            tc.tile_pool(name="osb", bufs=6) as osb,
            tc.tile_pool(name="otp", bufs=4) as otp,
            tc.tile_pool(name="rsp", bufs=8) as rsp,
            tc.tile_pool(name="yp", bufs=6) as yp,
            tc.tile_pool(name="psBig", bufs=2, space="PSUM") as psBig,
            tc.tile_pool(name="psSt", bufs=2, space="PSUM") as psSt,
            tc.tile_pool(name="psM", bufs=2, space="PSUM") as psM,
            tc.tile_pool(name="psO", bufs=2, space="PSUM") as psO,
        ):
            ident = const.tile([128, 128], BF16)
            make_identity(nc, ident)

            # multiplicative 0/1 causal mask for PT [ts, tq], tq-packed as
            # [ts0 x tq0 | ts0 x tq1 | ts1 x tq1]: keep ts <= tq, so the
            # outer blocks are triangular (keep col >= partition), middle
            # block is all-ones
            m0 = const.tile([128, 128], BF16)
            nc.gpsimd.memset(m0, 1.0)
            nc.gpsimd.affine_select(
                out=m0,
                in_=m0,
                compare_op=mybir.AluOpType.is_ge,
                fill=0.0,
                base=0,
                pattern=[[1, 128]],
                channel_multiplier=-1,
            )

            # input DMAs, critical-first: the first projection chunks need
            # wq + xt[g0], then wk, wv; weights issue on the sync queue while
            # xt prefetches issue in parallel on the (idle) gpsimd queue
            w_sb = {}
            for name, dram in (("wq", wqt), ("wk", wkt), ("wv", wvt), ("wo", wot)):
                w_sb[name] = const.tile([128, 3, C], BF16, tag=name, name=name)
            xts = [
                xp.tile([128, 3, GB, T], BF16, tag="x", name=f"xt{g}")
                for g in range(NG)
            ]
            w_drams = {"wq": wqt, "wk": wkt, "wv": wvt, "wo": wot}

            def load_w(name):
                nc.sync.dma_start(
                    out=w_sb[name],
                    in_=w_drams[name].rearrange("(k p) d -> p k d", p=128),
                )

            def load_x(g):
                for bi in range(GB):
                    nc.sync.dma_start(
                        out=xts[g][:, :, bi, :],
                        in_=xT[g * GB + bi].rearrange("(k p) t -> p k t", p=128),
                    )

            load_w("wq")
            load_x(0)
            load_w("wk")
            load_w("wv")
            load_x(1)
            load_w("wo")
            for g in range(2, NG):
                load_x(g)

            # per-group projection pre-work, chunked as a generator so it can
            # be interleaved into the previous group's attention units
            group_state = {}

            def gen_prework(g):
                # chunk order matters: scores of unit u need only the d-fold
                # pair u//2 of qt/kt, and the first AV needs v2[b0]; yield in
                # [qt0, kt0, v(b0,0), v(b0,1), qt1, kt1, qt2, kt2, v(b1,*)]
                # order so group 0 can start attention after two chunks
                xt = xts[g]
                qt_t = qkv.tile([128, 3, GB, T], BF16, tag="qt", name=f"qt{g}")
                kt_t = qkv.tile([128, 3, GB, T], BF16, tag="kt", name=f"kt{g}")
                v2s = [None, None]
                group_state[g] = (qt_t, kt_t, v2s)

                def qk_chunk(t, wname, d):
                    w = w_sb[wname]
                    ps = psBig.tile([128, GB * T], FP32, tag="big", name="qkps")
                    for k in range(3):
                        nc.tensor.matmul(
                            ps,
                            lhsT=w[:, k, d * 128 : (d + 1) * 128],
                            rhs=xt[:, k, :, :],
                            start=(k == 0),
                            stop=(k == 2),
                        )
                    nc.scalar.copy(t[:, d, :, :], ps)

                def v_chunk(bi, t2):
                    if t2 == 0:
                        v2s[bi] = vp.tile(
                            [128, 2, H, 65], BF16, tag="v", name="v2"
                        )
                        nc.gpsimd.memset(v2s[bi][:, :, :, 64:65], 1.0)
                    ps = psBig.tile([128, C], FP32, tag="big", name="vps")
                    for k in range(3):
                        nc.tensor.matmul(
                            ps,
                            lhsT=xt[:, k, bi, t2 * 128 : (t2 + 1) * 128],
                            rhs=w_sb["wv"][:, k, :],
                            start=(k == 0),
                            stop=(k == 2),
                        )
                    nc.vector.tensor_copy(v2s[bi][:, t2, :, 0:64], ps)

                qk_chunk(qt_t, "wq", 0)
                yield
                qk_chunk(kt_t, "wk", 0)
                yield
                for t2 in range(2):
                    v_chunk(0, t2)
                    yield
                for d in range(1, 3):
                    qk_chunk(qt_t, "wq", d)
                    yield
                    qk_chunk(kt_t, "wk", d)
                    yield
                for t2 in range(2):
                    v_chunk(1, t2)
                    yield

            pending = {}
            cur_osb = {}
            o2_state = {}

            def emit_scores(u):
                g, bi, h = u
                qt, kt, _ = group_state[g]
                pair, doff = h // 2, (h % 2) * 64
                qh = qt[doff : doff + 64, pair, bi, :]
                kh = kt[doff : doff + 64, pair, bi, :]
                st = psSt.tile([128, 384], FP32, tag="st", name="st")
                nc.tensor.matmul(
                    st[:, 0:256], lhsT=kh[:, 0:128], rhs=qh, start=True, stop=True
                )
                nc.tensor.matmul(
                    st[:, 256:384],
                    lhsT=kh[:, 128:256],
                    rhs=qh[:, 128:256],
                    start=True,
                    stop=True,
                )
                pt_t = pp.tile([128, 384], BF16, tag="pt", name="pt")
                nc.scalar.activation(pt_t, st, AFT.Exp, scale=SCALE)
                nc.vector.tensor_mul(pt_t[:, 0:128], pt_t[:, 0:128], m0)
                nc.gpsimd.tensor_mul(pt_t[:, 256:384], pt_t[:, 256:384], m0)
                pending[u] = pt_t

            def emit_av(u):
                g, bi, h = u
                _, _, v2s = group_state[g]
                pt_t = pending.pop(u)
                v2 = v2s[bi]
                if h % 2 == 0:
                    o2_state["t"] = psO.tile(
                        [128, 2, 2, 65], FP32, tag="o", name="o2"
                    )
                o2 = o2_state["t"]
                o = o2[:, h % 2]
                nc.tensor.matmul(
                    o[:, 0, :],
                    lhsT=pt_t[:, 0:128],
                    rhs=v2[:, 0, h, :],
                    start=True,
                    stop=True,
                )
                nc.tensor.matmul(
                    o[:, 1, :],
                    lhsT=pt_t[:, 128:256],
                    rhs=v2[:, 0, h, :],
                    start=True,
                    stop=False,
                )
                nc.tensor.matmul(
                    o[:, 1, :],
                    lhsT=pt_t[:, 256:384],
                    rhs=v2[:, 1, h, :],
                    start=False,
                    stop=True,
                )
                if h == 0:
                    cur_osb[bi] = osb.tile([128, 2, C], BF16, tag="osb", name="ob")
                ob = cur_osb[bi]
                if h % 2 == 1:
                    # one reciprocal + four scaled PSUM->SBUF moves for the
                    # completed head pair (scale is per-partition tq)
                    rs = rsp.tile([128, 4], FP32, tag="rs", name="rs")
                    nc.vector.reciprocal(rs, o2[:, :, :, 64:65])
                    for hp in range(2):
                        hh = h - 1 + hp
                        for t2 in range(2):
                            nc.vector.tensor_scalar_mul(
                                ob[:, t2, hh * 64 : (hh + 1) * 64],
                                o2[:, hp, t2, 0:64],
                                rs[:, 2 * hp + t2 : 2 * hp + t2 + 1],
                            )

            def emit_tail(g, bi):
                # transpose O [tq, D] -> OT [D, tq], then y = OT^T @ WoT
                ob = cur_osb[bi]
                ot = otp.tile([128, 3, T], BF16, tag="ot", name="ot")
                for t2 in range(2):
                    tps = psM.tile([128, 384], BF16, tag="m", name="tps")
                    for db in range(3):
                        nc.tensor.transpose(
                            tps[:, db * 128 : (db + 1) * 128],
                            ob[:, t2, db * 128 : (db + 1) * 128],
                            ident,
                        )
                    nc.vector.tensor_copy(ot[:, :, t2 * 128 : (t2 + 1) * 128], tps)
                ys = yp.tile([128, 2, C], BF16, tag="y", name="ys")
                for t2 in range(2):
                    ps = psM.tile([128, C], FP32, tag="m", name="ops")
                    for k in range(3):
                        nc.tensor.matmul(
                            ps[:, 0:C],
                            lhsT=ot[:, k, t2 * 128 : (t2 + 1) * 128],
                            rhs=w_sb["wo"][:, k, :],
                            start=(k == 0),
                            stop=(k == 2),
                        )
                    if t2 == 0:
                        nc.scalar.copy(ys[:, 0, :], ps[:, 0:C])
                    else:
                        nc.vector.tensor_copy(ys[:, 1, :], ps[:, 0:C])
                nc.sync.dma_start(
                    out=y[g * GB + bi].rearrange("(t2 p) c -> p t2 c", p=128),
                    in_=ys,
                )

            gens = [gen_prework(g) for g in range(NG)]
            for _ in gens[0]:  # group 0 pre-work as prologue
                pass
            units = [
                (g, bi, h) for g in range(NG) for bi in range(GB) for h in range(H)
            ]
            SKEW = 3  # units by which AV/normalize lag scores/exp/mask
            for i in range(len(units) + SKEW):
                if i >= SKEW and i - SKEW < len(units):
                    # next group's projection chunk first, so its last chunk
                    # lands before that group's first scores are emitted
                    gnext = units[i - SKEW][0] + 1
                    if gnext < NG:
                        next(gens[gnext], None)
                if i < len(units):
                    emit_scores(units[i])
                if i >= SKEW:
                    u = units[i - SKEW]
                    emit_av(u)
                    if u[2] == H - 1:
                        emit_tail(u[0], u[1])
    return nc


_NC = None


def _get_nc():
    global _NC
    if _NC is None:
        _NC = split_multi_waits(build_kernel())
    return _NC


def kernel(x, Wq, Wk, Wv, Wo, _trace=False):
    bf16 = ml_dtypes.bfloat16
    wq_t = np.ascontiguousarray(Wq.T).astype(bf16)
    wk_t = np.ascontiguousarray(Wk.T).astype(bf16)
    wv_t = np.ascontiguousarray(Wv.T).astype(bf16)
    wo_t = np.ascontiguousarray(Wo.T).astype(bf16)
    in_maps = []
    for i in range(N_CORES):
        xs = x[i * BL : (i + 1) * BL]  # [BL, T, C]
        xs_t = np.ascontiguousarray(xs.transpose(0, 2, 1)).astype(bf16)
        in_maps.append(
            {"xT": xs_t, "wqt": wq_t, "wkt": wk_t, "wvt": wv_t, "wot": wo_t}
        )
    res = run_bass_kernel_spmd(
        _get_nc(), in_maps, list(range(N_CORES)), trace=_trace
    )
    out = np.concatenate([r["y"] for r in res.results], axis=0)
    if _trace:
        return out.astype(np.float32), res
    return out.astype(np.float32)
